# revision 6
# baseline (speedup 1.0000x reference)
"""DOMINO++ loss kernel for Trainium2 (8 NeuronCores, data-parallel).

Strategy (v3)
-------------
Shard the (n=2, c=12, 96^3) logits over 8 cores: 4 contiguous spatial
blocks per batch element.  Each core reduces its 221184 voxels to a
[104, 96] PSUM block + per-chunk log-denominator accumulators; the host
combines the tiny per-core outputs into the scalar loss.

Host-side input encoding (layout/dtype only, no float math):
  - x ships fp8 in chunk layout [NCH, P, G, C, JB] (exp input).
  - weights ship fp8 as [NCH, P, G, 104]: cols 0:96 = one-hot of the
    target (PE stationary weights), cols 96:104 = host-GATHERED target
    logits x[v, tgt(v)] (a pure indexing op).  Since sum_c g_c = 1 per
    voxel, PSUM rows 96:104 yield sum_v x_tgt(v) with NO extra matmul
    and no extra moving columns.

Per-chunk device pipeline (chunks of FC=432 voxel-cols):
  DMA : x chunk + w chunk on the SP hwdge queue (Pool stays compute-only)
  ACT : y = Exp(x)
  DVE : t6/t3 pairwise tree (bf16 stride-1 => 2x mode)
  POOL: dna = t3_0+t3_1, dall = dna+t3_2 (bf16), rb = 1/dall
  DVE : g = y * rb  -> gt (bf16)
  PE  : per group g: ldweights w[128,104] (fp8), one matmul with
        moving = g [128, 96], accumulating into PSUM [104, 96]:
          rows (t,j): sum_v m_t g_c   (dice/penalty/CE terms)
          rows 96+j : sum_v x_tgt g_c (CE target-logit row)
Tail: Ln over the accumulated denominators (accum -> logd), PSUM ->
      SBUF copy, DMA out.  Two activation-table loads per run.
"""

import os
import sys
from contextlib import ExitStack

import numpy as np

sys.path.insert(0, "/opt/trn_rl_repo")

from concourse import bacc, bass, mybir, tile  # noqa: E402
from concourse import bass_utils  # noqa: E402

F32 = mybir.dt.float32
BF16 = mybir.dt.bfloat16
FP8 = mybir.dt.float8e4
ALU = mybir.AluOpType
ACTF = mybir.ActivationFunctionType

N_CORES = 8
C = 12            # classes
P = 128           # SBUF partitions
FT = 1728         # free size per partition per core (P*FT = 221184 voxels)
NCH = 4           # chunks
FC = FT // NCH    # voxel-columns per chunk (432)
JB = 8            # voxel-columns per matmul group (12*JB <= 128)
G = FC // JB      # matmul groups per chunk (54)
W = C * JB + JB   # weight columns per group (104: mask 96 + xtgt 8)
S = P * FT        # voxels per core
N, H, Wd, Z = 2, 96, 96, 96
SPATIAL = H * Wd * Z         # 884736 voxels per batch element
CORES_PER_N = N_CORES // N   # 4
CJ = C * JB                  # 96

_CACHE = {}

# Pool's ISA has no divide op, so the reciprocal stays on DVE
_RECIP_ON_DVE = True


def _build_program():
    """Build + compile the per-core Bass program (identical on all cores)."""
    nc = bacc.Bacc("TRN2", target_bir_lowering=False, debug=False,
                   num_devices=N_CORES)

    x_d = nc.dram_tensor("x", (NCH, P, C * FC), FP8, kind="ExternalInput")
    w_d = nc.dram_tensor("w", (NCH, P, G * W), FP8, kind="ExternalInput")
    # output: [0:104, 0:96] = psum, [:, 96:98] = logd accums
    out_d = nc.dram_tensor("m_out", (P, CJ + 2), F32, kind="ExternalOutput")

    with ExitStack() as ctx:
        tc = ctx.enter_context(tile.TileContext(nc))
        sb = ctx.enter_context(tc.tile_pool(name="sb", bufs=3))
        acc = ctx.enter_context(tc.tile_pool(name="acc", bufs=1))
        ps = ctx.enter_context(tc.tile_pool(name="ps", bufs=1, space="PSUM"))

        dall = acc.tile([P, NCH, FC], BF16)      # per-chunk denominators
        psum = ps.tile([W, CJ], F32)
        msb = acc.tile([P, CJ + 2], F32)         # combined output staging
        # partition starts must be 32-aligned; rows 96:104 are overwritten
        # by the PSUM copy afterwards
        nc.vector.memset(msb[CJ:, :CJ], 0.0)

        from concourse.dve_ops import (RECIP_APPROX_FAST_CONSTS,
                                       RECIPROCAL_APPROX_FAST)

        for ch in range(NCH):
            xt = sb.tile([P, C * FC], FP8, tag="xt", name=f"xt{ch}")
            wt = sb.tile([P, G, W], FP8, tag="wt", name=f"wt{ch}")
            gt = sb.tile([P, C * FC], BF16, tag="gt", name=f"gt{ch}")
            yt = sb.tile([P, C * FC], BF16, tag="yt", name=f"yt{ch}")
            t6 = sb.tile([P, G, 6, JB], BF16, tag="t6", name=f"t6_{ch}")
            t3 = sb.tile([P, G, 3, JB], BF16, tag="t3", name=f"t3_{ch}")
            dna = sb.tile([P, FC], BF16, tag="dna", name=f"dna{ch}")
            rb = sb.tile([P, FC], BF16, tag="rb", name=f"rb{ch}")

            nc.sync.dma_start(xt[:], x_d[ch])
            nc.sync.dma_start(wt[:].rearrange("p g w -> p (g w)"), w_d[ch])

            nc.scalar.activation(yt[:], xt[:], ACTF.Exp)

            # denominator: pairwise tree over the class dim (stride-1 inner)
            y4 = yt[:].rearrange("p (g c j) -> p g c j", g=G, j=JB)
            nc.vector.tensor_tensor(t6[:], y4[:, :, 0::2], y4[:, :, 1::2],
                                    op=ALU.add)
            nc.vector.tensor_tensor(t3[:], t6[:, :, 0::2], t6[:, :, 1::2],
                                    op=ALU.add)
            with nc.allow_low_precision(reason="bf16 softmax denominators"):
                nc.gpsimd.tensor_tensor(
                    dna[:].rearrange("p (g j) -> p g j", j=JB),
                    t3[:, :, 0], t3[:, :, 1], op=ALU.add)
                nc.gpsimd.tensor_tensor(
                    dall[:, ch].rearrange("p (g j) -> p g j", j=JB),
                    dna[:].rearrange("p (g j) -> p g j", j=JB),
                    t3[:, :, 2], op=ALU.add)

                cc = RECIP_APPROX_FAST_CONSTS
                nc.vector._custom_dve(RECIPROCAL_APPROX_FAST, out=rb[:],
                                      in0=dall[:, ch], s0=cc["s0"],
                                      s1=cc["s1"], imm2=cc["imm2"])

            rb_b = rb[:].rearrange("p (g j) -> p g () j", j=JB) \
                .to_broadcast([P, G, C, JB])
            nc.vector.tensor_tensor(
                gt[:].rearrange("p (g c j) -> p g c j", g=G, j=JB),
                y4, rb_b, op=ALU.mult)

            gt4 = gt[:].rearrange("p (g c j) -> p g c j", g=G, j=JB)
            for g in range(G):
                nc.tensor.matmul(psum[:], wt[:, g], gt4[:, g],
                                 start=(ch == 0 and g == 0),
                                 stop=(ch == NCH - 1 and g == G - 1))

        # logd in two pieces; both hide under the last chunk's DVE/PE work
        d0 = dall[:, :NCH - 1].rearrange("p ch f -> p (ch f)")
        nc.scalar.activation(d0, d0, ACTF.Ln, accum_out=msb[:, CJ:CJ + 1])
        d1 = dall[:, NCH - 1:].rearrange("p ch f -> p (ch f)")
        nc.scalar.activation(d1, d1, ACTF.Ln, accum_out=msb[:, CJ + 1:])
        nc.vector.tensor_copy(msb[:W, :CJ], psum[:])
        nc.sync.dma_start(out_d[:], msb[:])

    nc.compile()
    return nc


def _get_program():
    if "nc" not in _CACHE:
        _CACHE["nc"] = _build_program()
    return _CACHE["nc"]


def _shard_inputs(input, target):
    """Full inputs -> 8 per-core in_maps: x chunks + (mask|xtgt) weights."""
    fp8 = mybir.dt.np(FP8)
    x = np.asarray(input, dtype=np.float32)
    tg = np.asarray(target).reshape(N, SPATIAL).astype(np.int32)
    eye = np.eye(C, dtype=np.float32)
    in_maps = []
    for k in range(N_CORES):
        n = k // CORES_PER_N
        o = (k % CORES_PER_N) * S
        xn = x[n].reshape(C, SPATIAL)[:, o:o + S]        # [C, S]
        # voxel v = (ch, p, g, j); class dim interposed: [NCH, P, G, C, JB]
        xs = xn.reshape(C, NCH, P, G, JB).transpose(1, 2, 3, 0, 4) \
            .reshape(NCH, P, C * FC)
        ts = tg[n, o:o + S].reshape(NCH, P, G, JB)
        ms = eye[ts].transpose(0, 1, 2, 4, 3)            # [NCH,P,G,C,JB]
        # host gather of the target logit per voxel (indexing only)
        xt = np.take_along_axis(xn, tg[n, o:o + S][None], axis=0)[0] \
            .reshape(NCH, P, G, JB)
        w = np.concatenate(
            [ms.reshape(NCH, P, G, CJ), xt], axis=-1)    # [NCH,P,G,104]
        in_maps.append({"x": np.ascontiguousarray(xs).astype(fp8),
                        "w": np.ascontiguousarray(w.reshape(NCH, P, G * W))
                        .astype(fp8)})
    return in_maps


def _combine(results, matrix_penalty, global_step, maxiter):
    pen = np.asarray(matrix_penalty, dtype=np.float64)
    inter = np.zeros((N, C))
    ground = np.zeros((N, C))
    pred = np.zeros((N, C))
    xtgt_sum = 0.0
    logd_sum = 0.0
    pen_sum = 0.0
    for k, r in enumerate(results):
        n = k // CORES_PER_N
        out = np.asarray(r["m_out"], dtype=np.float64)
        mfull = out[:CJ, :CJ].reshape(C, JB, C, JB)
        mg = np.einsum("tjcj->tc", mfull)            # sum_v m_t * g_c
        inter[n] += np.diag(mg)
        ground[n] += mg.sum(axis=1)                  # masks partition unity
        pred[n] += mg.sum(axis=0)
        xrows = out[CJ:W, :CJ].reshape(JB, C, JB)
        xtgt_sum += np.einsum("jcj->", xrows)        # sum_c at j'=j
        logd_sum += float(out[:, CJ:].sum())
        pen_sum += float((pen * mg).sum())

    nvox = N * SPATIAL
    dice = 1.0 - (2.0 * inter + 1e-5) / (ground + pred + 1e-5)
    dice_loss = dice.mean()
    ce = (logd_sum - xtgt_sum) / nvox
    ce_total = dice_loss + ce
    pen_mean = pen_sum / nvox
    beta = 10.0 ** np.floor(np.log10(ce_total))
    gs = float(global_step)
    mi = float(maxiter)
    alpha0 = 1.0 - gs / mi
    alpha1 = gs / mi
    return np.float32(alpha1 * ce_total + alpha0 * beta * pen_mean)


def kernel(input, target, matrix_penalty, global_step, maxiter):
    nc = _get_program()
    in_maps = _shard_inputs(input, target)
    trace = bool(int(os.environ.get("BASS_LOSS_TRACE", "0")))
    res = bass_utils.run_bass_kernel_spmd(
        nc, in_maps, core_ids=list(range(N_CORES)), trace=trace)
    _CACHE["last_exec_ns"] = res.exec_time_ns
    return _combine(res.results, matrix_penalty, global_step, maxiter)
